# revision 2
# baseline (speedup 1.0000x reference)
"""Trainium2 Bass kernel for nn_DecoderLayer (causal linear self-attn +
linear cross-attn + FFN decoder layer), SPMD over 8 NeuronCores.

Single fused NEFF launch. Sharding: tokens split 8 ways — core c handles
batch c//4, token quarter c%4 (512 tokens); weights replicated. The two
sequence-global reductions (causal-attention prefix state S and
cross-attention context sums) are exchanged on-device with two small
AllGathers over groups [[0..3],[4..7]]; the exclusive prefix over S is
realized with a per-core host-constant mask (pmask) so the SPMD program
stays branch-free.

Precision: projections/FFN run bf16 on the PE with fp32 PSUM
accumulation (weights cast host-side); attention internals bf16.
"""

import sys

sys.path.insert(0, "/opt/trn_rl_repo")

import numpy as np

import concourse.bass as bass
import concourse.mybir as mybir
import concourse.tile as tile
from concourse import bacc, bass_utils
from concourse.masks import make_identity, make_upper_triangular

F32 = mybir.dt.float32
F32R = mybir.dt.float32r
BF16 = mybir.dt.bfloat16
AF = mybir.ActivationFunctionType
ALU = mybir.AluOpType
AX = mybir.AxisListType

P = 128
B, N, DIM, H, FF = 2, 2048, 512, 8, 2048
E = DIM // H  # 64
E1 = E + 1  # 65 ([value | ones] column)
T = N // 4  # 512 tokens per core
NT = T // P  # 4 token tiles per core
KF = DIM // P  # 4 feature tiles
NFF = FF // P  # 16
LN_EPS = 1e-5
ATTN_EPS = 1e-6
QS_SCALE = float(E) ** -0.5  # 0.125
N_CORES = 8
REPLICA_GROUPS = [[0, 1, 2, 3], [4, 5, 6, 7]]


class Evict:
    """Round-robin PSUM->SBUF eviction across DVE and ACT to balance load."""

    def __init__(self, nc):
        self.nc = nc
        self.i = 0

    def copy(self, out, in_):
        if self.i % 2 == 0:
            self.nc.vector.tensor_copy(out, in_)
        else:
            self.nc.scalar.copy(out, in_)
        self.i += 1

    def add(self, out, in0, in1):
        self.nc.vector.tensor_add(out, in0, in1)
        self.i += 1


def _layernorm(nc, sbuf, eps_tile, x3, g_bc, b_bc, out3, trivial):
    """LayerNorm over the feature (free) axis, batched across the NT
    token tiles of a (P, NT, DIM) input where possible."""
    ssum = sbuf.tile([P, NT], F32, name="ln_ssum")
    nc.vector.reduce_sum(ssum[:], x3[:], axis=AX.X)
    negmu = sbuf.tile([P, NT], F32, name="ln_negmu")
    nc.scalar.mul(negmu[:], ssum[:], -1.0 / DIM)
    ssq = sbuf.tile([P, NT], F32, name="ln_ssq")
    for mt in range(NT):
        sq = sbuf.tile([P, DIM], F32, name="ln_sq")
        nc.scalar.activation(sq[:], x3[:, mt, :], AF.Square,
                             bias=negmu[:, mt:mt + 1],
                             accum_out=ssq[:, mt:mt + 1])
    std = sbuf.tile([P, NT], F32, name="ln_std")
    nc.scalar.activation(std[:], ssq[:], AF.Sqrt, bias=eps_tile[:],
                         scale=1.0 / DIM)
    rstd = sbuf.tile([P, NT], F32, name="ln_rstd")
    nc.vector.reciprocal(rstd[:], std[:])
    for mt in range(NT):
        if trivial:
            nc.vector.tensor_scalar(out3[:, mt, :], x3[:, mt, :],
                                    negmu[:, mt:mt + 1], rstd[:, mt:mt + 1],
                                    ALU.add, ALU.mult)
        else:
            xh = sbuf.tile([P, DIM], F32, name="ln_xh")
            nc.vector.tensor_scalar(xh[:], x3[:, mt, :],
                                    negmu[:, mt:mt + 1], rstd[:, mt:mt + 1],
                                    ALU.add, ALU.mult)
            xg = sbuf.tile([P, DIM], F32, name="ln_xg")
            nc.vector.tensor_mul(xg[:], xh[:], g_bc[:])
            nc.vector.tensor_add(out3[:, mt, :], xg[:], b_bc[:])


def _dma_bcast(nc, pool, dram_ap, width, name):
    t = pool.tile([P, width], F32, name=name)
    nc.sync.dma_start(t[:], dram_ap[None, :].to_broadcast((P, width)))
    return t


def build_fused(trivial):
    nc = bacc.Bacc(None, target_bir_lowering=False, debug=False,
                   num_devices=N_CORES)
    x_d = nc.dram_tensor("x", [T, DIM], F32, kind="ExternalInput")
    mem_d = nc.dram_tensor("mem", [T, DIM], F32, kind="ExternalInput")
    pmask_d = nc.dram_tensor("pmask", [4], F32, kind="ExternalInput")
    wqvk_d = nc.dram_tensor("W_qvk", [DIM, 3 * DIM], BF16, kind="ExternalInput")
    wkv_d = nc.dram_tensor("W_kv", [DIM, 2 * DIM], BF16, kind="ExternalInput")
    wq_d = nc.dram_tensor("W_q", [DIM, DIM], BF16, kind="ExternalInput")
    wff1_d = nc.dram_tensor("W_ff1", [DIM, FF], BF16, kind="ExternalInput")
    wff2_d = nc.dram_tensor("W_ff2", [FF, DIM], BF16, kind="ExternalInput")
    bff1_d = nc.dram_tensor("b_ff1", [FF], F32, kind="ExternalInput")
    lng = {}
    if not trivial:
        bqvk_d = nc.dram_tensor("b_qvk", [3 * DIM], F32, kind="ExternalInput")
        bkv_d = nc.dram_tensor("b_kv", [2 * DIM], F32, kind="ExternalInput")
        bq_d = nc.dram_tensor("b_q", [DIM], F32, kind="ExternalInput")
        bff2_d = nc.dram_tensor("b_ff2", [DIM], F32, kind="ExternalInput")
        for i in (1, 2, 3):
            lng[f"g{i}"] = nc.dram_tensor(f"ln{i}_g", [DIM], F32,
                                          kind="ExternalInput")
            lng[f"b{i}"] = nc.dram_tensor(f"ln{i}_b", [DIM], F32,
                                          kind="ExternalInput")
    out_d = nc.dram_tensor("out", [T, DIM], F32, kind="ExternalOutput")

    with tile.TileContext(nc) as tc:
        with (
            tc.tile_pool(name="const", bufs=1) as cpool,
            tc.tile_pool(name="acts", bufs=1) as acts,
            tc.tile_pool(name="w", bufs=1) as wpool,
            tc.tile_pool(name="sb", bufs=3) as sbuf,
            tc.tile_pool(name="pw1", bufs=2) as pw1,
            tc.tile_pool(name="ph1", bufs=3) as ph1,
            tc.tile_pool(name="pw2", bufs=2) as pw2,
            tc.tile_pool(name="pt", bufs=2, space="PSUM") as pp_t,
            tc.tile_pool(name="pb", bufs=4, space="PSUM") as pp_b,
            tc.tile_pool(name="ps", bufs=2, space="PSUM") as pp_s,
            tc.tile_pool(name="dram", bufs=4, space="DRAM") as dram,
        ):
            ev = Evict(nc)
            ident = cpool.tile([P, P], F32, name="ident")
            make_identity(nc, ident[:])
            ident_bf = cpool.tile([P, P], BF16, name="ident_bf")
            make_identity(nc, ident_bf[:])
            eps_t = cpool.tile([P, 1], F32, name="eps_t")
            nc.vector.memset(eps_t[:], LN_EPS)
            umask = cpool.tile([P, P], BF16, name="umask")
            make_upper_triangular(nc, umask[:], val=1.0, diag=True)
            pmask_bc = _dma_bcast(nc, cpool, pmask_d.ap(), 4, "pmask_bc")
            bff1T = cpool.tile([P, NFF], F32, name="bff1T")
            nc.sync.dma_start(bff1T[:], bff1_d.ap().rearrange("(m p) -> p m", p=P))
            g_bc = {k: None for k in ("g1", "b1", "g2", "b2", "g3", "b3")}
            if not trivial:
                bqvk_bc = _dma_bcast(nc, cpool, bqvk_d.ap(), 3 * DIM, "bqvk_bc")
                bkv_bc = _dma_bcast(nc, cpool, bkv_d.ap(), 2 * DIM, "bkv_bc")
                bq_bc = _dma_bcast(nc, cpool, bq_d.ap(), DIM, "bq_bc")
                bff2_bc = _dma_bcast(nc, cpool, bff2_d.ap(), DIM, "bff2_bc")
                for i in (1, 2, 3):
                    g_bc[f"g{i}"] = _dma_bcast(nc, cpool, lng[f"g{i}"].ap(),
                                               DIM, f"g{i}bc")
                    g_bc[f"b{i}"] = _dma_bcast(nc, cpool, lng[f"b{i}"].ap(),
                                               DIM, f"b{i}bc")

            # ------------- input DMAs (issue order = DMA priority) -------
            xn = acts.tile([P, NT, DIM], F32, name="xn")
            nc.sync.dma_start(xn[:], x_d.ap().rearrange("(m p) n -> p m n", p=P))
            wqvk = wpool.tile([P, KF, 3 * DIM], BF16, name="wqvk")
            nc.sync.dma_start(wqvk[:], wqvk_d.ap().rearrange("(k p) n -> p k n", p=P))
            memn = acts.tile([P, NT, DIM], F32, name="memn")
            nc.sync.dma_start(memn[:], mem_d.ap().rearrange("(m p) n -> p m n", p=P))
            wkv = wpool.tile([P, KF, 2 * DIM], BF16, name="wkv")
            nc.sync.dma_start(wkv[:], wkv_d.ap().rearrange("(k p) n -> p k n", p=P))
            # W_q shares the FFN-w1 streaming pool (same 8KB/partition shape)
            wq = pw1.tile([P, KF, DIM], BF16, name="wq", tag="w1s")
            nc.sync.dma_start(wq[:], wq_d.ap().rearrange("(k p) n -> p k n", p=P))

            # ------------- x transpose + qvk projection ------------------
            xT = acts.tile([P, KF, T], BF16, name="xT")
            for kf in range(KF):
                for mt in range(NT):
                    pt = pp_t.tile([P, P], F32, name="tpsum", tag="t128")
                    nc.tensor.transpose(pt[:], xn[:, mt, kf * P:(kf + 1) * P],
                                        ident[:])
                    ev.copy(xT[:, kf, mt * P:(mt + 1) * P], pt[:])

            # qvk evictions go straight from PSUM into attention form:
            # q -> exp(q) (softmax numerator), k -> exp(k), v -> [v|1] bf16
            qs_bf = acts.tile([P, NT, DIM], BF16, name="qs_bf")
            ke_bf = acts.tile([P, NT, DIM], BF16, name="ke_bf")
            v1 = acts.tile([P, NT, H, E1], BF16, name="v1")
            nc.gpsimd.memset(v1[:], 1.0)
            for mt in range(NT):
                for nb in range(3):
                    ps = pp_b.tile([P, 512], F32, name="proj_ps", tag="proj")
                    for kf in range(KF):
                        nc.tensor.matmul(
                            ps[:], xT[:, kf, mt * P:(mt + 1) * P],
                            wqvk[:, kf, nb * 512:(nb + 1) * 512],
                            start=(kf == 0), stop=(kf == KF - 1),
                        )
                    if not trivial:
                        pst = sbuf.tile([P, 512], F32, name="bias_t")
                        nc.vector.tensor_add(
                            pst[:], ps[:], bqvk_bc[:, nb * 512:(nb + 1) * 512])
                        src = pst[:]
                    else:
                        src = ps[:]
                    if nb == 0:  # q -> exp -> per-head softmax -> bf16
                        qe = sbuf.tile([P, H, E], F32, name="sm_qe")
                        nc.scalar.activation(
                            qe[:], src.rearrange("p (h e) -> p h e", e=E),
                            AF.Exp)
                        qsum = sbuf.tile([P, H], F32, name="sm_qsum")
                        nc.vector.reduce_sum(qsum[:], qe[:], axis=AX.X)
                        qrec = sbuf.tile([P, H], F32, name="sm_qrec")
                        nc.vector.reciprocal(qrec[:], qsum[:])
                        nc.vector.tensor_scalar_mul(qrec[:], qrec[:], QS_SCALE)
                        nc.vector.tensor_mul(
                            qs_bf[:, mt, :].rearrange("p (h e) -> p h e", e=E),
                            qe[:],
                            qrec[:, :, None].to_broadcast((P, H, E)),
                        )
                    elif nb == 1:  # v -> [v|1]
                        nc.scalar.copy(
                            v1[:, mt, :, 0:E],
                            src.rearrange("p (h e) -> p h e", e=E))
                    else:  # k -> exp(k)
                        nc.scalar.activation(ke_bf[:, mt, :], src, AF.Exp)

            # ------------- per-chunk self-attn sums S + AllGather --------
            s_sb = acts.tile([P, NT, KF, E1], BF16, name="s_sb")
            for j in range(NT):
                for hp in range(KF):
                    ps = pp_s.tile([P, E1], F32, name="s_ps", tag="small")
                    for i in range(2):
                        h = 2 * hp + i
                        nc.tensor.matmul(ps[i * E:(i + 1) * E, :],
                                         ke_bf[:, j, h * E:(h + 1) * E],
                                         v1[:, j, h, :],
                                         start=True, stop=True)
                    ev.copy(s_sb[:, j, hp, :], ps[:])
            st01 = sbuf.tile([P, KF, E1], F32, name="st01")
            nc.vector.tensor_add(st01[:], s_sb[:, 0], s_sb[:, 1])
            st23 = sbuf.tile([P, KF, E1], F32, name="st23")
            nc.vector.tensor_add(st23[:], s_sb[:, 2], s_sb[:, 3])
            s_tot_bf = sbuf.tile([P, KF, E1], BF16, name="s_tot_bf")
            nc.vector.tensor_add(s_tot_bf[:], st01[:], st23[:])
            bs_in = dram.tile([KF * P, E1], BF16)
            nc.gpsimd.dma_start(bs_in[:].rearrange("(s p) n -> p s n", p=P),
                                s_tot_bf[:])
            bs_out = dram.tile([4 * KF * P, E1], BF16)
            nc.gpsimd.collective_compute(
                "AllGather", ALU.bypass, replica_groups=REPLICA_GROUPS,
                ins=[bs_in[:].opt()], outs=[bs_out[:].opt()],
            )
            gath_s = acts.tile([P, 4, KF, E1], BF16, name="gath_s")
            nc.gpsimd.dma_start(
                gath_s[:],
                bs_out[:].rearrange("(c s p) n -> p c s n", p=P, s=KF))

            # ------------- mem transpose + kv projection -----------------
            memT = acts.tile([P, KF, T], BF16, name="memT")
            for kf in range(KF):
                for mt in range(NT):
                    pt = pp_t.tile([P, P], F32, name="tpsum", tag="t128")
                    nc.tensor.transpose(pt[:], memn[:, mt, kf * P:(kf + 1) * P],
                                        ident[:])
                    ev.copy(memT[:, kf, mt * P:(mt + 1) * P], pt[:])
            kec_bf = acts.tile([P, NT, DIM], BF16, name="kec_bf")
            v1c = acts.tile([P, NT, H, E1], BF16, name="v1c")
            nc.gpsimd.memset(v1c[:], 1.0)
            for mt in range(NT):
                for nb in range(2):
                    ps = pp_b.tile([P, 512], F32, name="proj_ps", tag="proj")
                    for kf in range(KF):
                        nc.tensor.matmul(
                            ps[:], memT[:, kf, mt * P:(mt + 1) * P],
                            wkv[:, kf, nb * 512:(nb + 1) * 512],
                            start=(kf == 0), stop=(kf == KF - 1),
                        )
                    if not trivial:
                        pst = sbuf.tile([P, 512], F32, name="bias_t")
                        nc.vector.tensor_add(
                            pst[:], ps[:], bkv_bc[:, nb * 512:(nb + 1) * 512])
                        src = pst[:]
                    else:
                        src = ps[:]
                    if nb == 0:  # k -> exp(k)
                        nc.scalar.activation(kec_bf[:, mt, :], src, AF.Exp)
                    else:  # v -> [v|1]
                        nc.vector.tensor_copy(
                            v1c[:, mt, :, 0:E],
                            src.rearrange("p (h e) -> p h e", e=E))

            # ------------- cross-attn context sums + AllGather -----------
            ctx_sb = acts.tile([P, KF, E1], BF16, name="ctx_sb")
            for hp in range(KF):
                ps = pp_s.tile([P, E1], F32, name="ctx_ps", tag="small")
                for i in range(2):
                    h = 2 * hp + i
                    for j in range(NT):
                        nc.tensor.matmul(ps[i * E:(i + 1) * E, :],
                                         kec_bf[:, j, h * E:(h + 1) * E],
                                         v1c[:, j, h, :],
                                         start=(j == 0), stop=(j == NT - 1))
                ev.copy(ctx_sb[:, hp, :], ps[:])
            bc_in = dram.tile([KF * P, E1], BF16)
            nc.gpsimd.dma_start(bc_in[:].rearrange("(s p) n -> p s n", p=P),
                                ctx_sb[:])
            bc_out = dram.tile([4 * KF * P, E1], BF16)
            nc.gpsimd.collective_compute(
                "AllGather", ALU.bypass, replica_groups=REPLICA_GROUPS,
                ins=[bc_in[:].opt()], outs=[bc_out[:].opt()],
            )
            gath_c = acts.tile([P, 4, KF, E1], BF16, name="gath_c")
            nc.gpsimd.dma_start(
                gath_c[:],
                bc_out[:].rearrange("(c s p) n -> p c s n", p=P, s=KF))

            # ------------- qs/ke transposes ------------------------------
            qsT = acts.tile([P, KF, NT, P], BF16, name="qsT")
            keT = acts.tile([P, KF, NT, P], BF16, name="keT")
            for hp in range(KF):
                for mt in range(NT):
                    pt = pp_t.tile([P, P], BF16, name="tp_bf", tag="t128")
                    nc.tensor.transpose(pt[:], qs_bf[:, mt, hp * P:(hp + 1) * P],
                                        ident_bf[:])
                    ev.copy(qsT[:, hp, mt, :], pt[:])
                    pt2 = pp_t.tile([P, P], BF16, name="tp_bf", tag="t128")
                    nc.tensor.transpose(pt2[:], ke_bf[:, mt, hp * P:(hp + 1) * P],
                                        ident_bf[:])
                    ev.copy(keT[:, hp, mt, :], pt2[:])

            # ------------- intra-chunk causal attention ------------------
            poi_sb = acts.tile([P, NT, H, E1], BF16, name="poi_sb")
            for j in range(NT):
                for g in range(2):
                    po4 = pp_s.tile([P, 4 * E1], F32, name="poi_ps",
                                    tag="small")
                    for i in range(4):
                        h = 4 * g + i
                        hp, prow = h // 2, (h % 2) * E
                        pa = pp_b.tile([P, P], F32, name="at_ps", tag="proj")
                        nc.tensor.matmul(pa[:], keT[prow:prow + E, hp, j, :],
                                         qsT[prow:prow + E, hp, j, :],
                                         start=True, stop=True)
                        amt = sbuf.tile([P, P], BF16, name="amt")
                        nc.vector.tensor_mul(amt[:], pa[:], umask[:])
                        nc.tensor.matmul(po4[:, i * E1:(i + 1) * E1], amt[:],
                                         v1[:, j, h, :], start=True, stop=True)
                    ev.copy(poi_sb[:, j, 4 * g:4 * g + 4, :],
                            po4[:].rearrange("p (h n) -> p h n", n=E1))

            # ------------- consume gathered S: exclusive core prefix -----
            p1 = sbuf.tile([P, KF, E1], BF16, name="p1", tag="p1")
            nc.vector.tensor_scalar_mul(p1[:], gath_s[:, 0],
                                        pmask_bc[:, 0:1])
            for c in range(1, 4):
                tmp = sbuf.tile([P, KF, E1], BF16, name="pc_t")
                nc.vector.tensor_scalar_mul(tmp[:], gath_s[:, c],
                                            pmask_bc[:, c:c + 1])
                p1n = sbuf.tile([P, KF, E1], BF16, name="p1", tag="p1")
                nc.vector.tensor_add(p1n[:], p1[:], tmp[:])
                p1 = p1n

            # ------------- inter-chunk attention + combine + LN1 ---------
            attn_n = acts.tile([P, NT, DIM], F32, name="attn_n")
            for j in range(NT):
                for g in range(2):
                    comb = sbuf.tile([P, 4, E1], F32, name="compo")
                    for i in range(4):
                        h = 4 * g + i
                        hp, prow = h // 2, (h % 2) * E
                        po = pp_s.tile([P, E1], F32, name="o_ps", tag="small")
                        nc.tensor.matmul(po[:],
                                         qsT[prow:prow + E, hp, j, :],
                                         p1[prow:prow + E, hp, :],
                                         start=True, stop=True)
                        nc.vector.tensor_add(comb[:, i, :], po[:],
                                             poi_sb[:, j, 4 * g + i, :])
                    den = sbuf.tile([P, 4], F32, name="den")
                    nc.scalar.activation(den[:], comb[:, :, E], AF.Copy,
                                         bias=ATTN_EPS * QS_SCALE)
                    dinv = sbuf.tile([P, 4], F32, name="dinv")
                    nc.vector.reciprocal(dinv[:], den[:])
                    nc.vector.tensor_mul(
                        attn_n[:, j, :].rearrange(
                            "p (h e) -> p h e", e=E)[:, 4 * g:4 * g + 4, :],
                        comb[:, :, 0:E],
                        dinv[:, :, None].to_broadcast((P, 4, E)),
                    )
                if j < NT - 1:
                    p1n = sbuf.tile([P, KF, E1], BF16, name="p1", tag="p1")
                    nc.vector.tensor_add(p1n[:], p1[:], s_sb[:, j])
                    p1 = p1n

            res1 = acts.tile([P, NT, DIM], F32, name="res1")
            for mt in range(NT):
                nc.vector.tensor_add(res1[:, mt, :], attn_n[:, mt, :],
                                     xn[:, mt, :])
            ln1_n = acts.tile([P, NT, DIM], F32, name="ln1_n")
            _layernorm(nc, sbuf, eps_t, res1, g_bc["g1"], g_bc["b1"], ln1_n,
                       trivial)
            ln1T = acts.tile([P, KF, T], BF16, name="ln1T", tag="xT")
            for kf in range(KF):
                for mt in range(NT):
                    pt = pp_t.tile([P, P], F32, name="tpsum", tag="t128")
                    nc.tensor.transpose(pt[:], ln1_n[:, mt, kf * P:(kf + 1) * P],
                                        ident[:])
                    ev.copy(ln1T[:, kf, mt * P:(mt + 1) * P], pt[:])

            # ------------- cross-attention: q projection + softmax -------
            qsc_bf = acts.tile([P, NT, DIM], BF16, name="qsc_bf", tag="qs_bf")
            for mt in range(NT):
                ps = pp_b.tile([P, 512], F32, name="proj_ps", tag="proj")
                for kf in range(KF):
                    nc.tensor.matmul(ps[:], ln1T[:, kf, mt * P:(mt + 1) * P],
                                     wq[:, kf, :],
                                     start=(kf == 0), stop=(kf == KF - 1))
                if not trivial:
                    pst = sbuf.tile([P, 512], F32, name="bias_t")
                    nc.vector.tensor_add(pst[:], ps[:], bq_bc[:])
                    src = pst[:]
                else:
                    src = ps[:]
                qe = sbuf.tile([P, H, E], F32, name="sm_qe")
                nc.scalar.activation(qe[:], src.rearrange("p (h e) -> p h e", e=E),
                                     AF.Exp)
                qsum = sbuf.tile([P, H], F32, name="sm_qsum")
                nc.vector.reduce_sum(qsum[:], qe[:], axis=AX.X)
                qrec = sbuf.tile([P, H], F32, name="sm_qrec")
                nc.vector.reciprocal(qrec[:], qsum[:])
                nc.vector.tensor_scalar_mul(qrec[:], qrec[:], QS_SCALE)
                nc.gpsimd.tensor_mul(
                    qsc_bf[:, mt, :].rearrange("p (h e) -> p h e", e=E),
                    qe[:],
                    qrec[:, :, None].to_broadcast((P, H, E)),
                )
            qscT = acts.tile([P, KF, NT, P], BF16, name="qscT", tag="qsT")
            for hp in range(KF):
                for mt in range(NT):
                    pt = pp_t.tile([P, P], BF16, name="tp_bf", tag="t128")
                    nc.tensor.transpose(pt[:], qsc_bf[:, mt, hp * P:(hp + 1) * P],
                                        ident_bf[:])
                    ev.copy(qscT[:, hp, mt, :], pt[:])

            # ------------- consume gathered ctx: normalize (block-diag) --
            c01 = sbuf.tile([P, KF, E1], F32, name="c01")
            nc.vector.tensor_add(c01[:], gath_c[:, 0], gath_c[:, 1])
            c23 = sbuf.tile([P, KF, E1], F32, name="c23")
            nc.vector.tensor_add(c23[:], gath_c[:, 2], gath_c[:, 3])
            csum = sbuf.tile([P, KF, E1], F32, name="csum")
            nc.vector.tensor_add(csum[:], c01[:], c23[:])
            crec = sbuf.tile([P, KF], F32, name="crec")
            nc.vector.reciprocal(crec[:], csum[:, :, E])
            ctxn_bd = acts.tile([P, KF, 2, E], BF16, name="ctxn_bd")
            nc.gpsimd.memset(ctxn_bd[:], 0.0)
            for s in range(KF):
                for i in range(2):
                    nc.vector.tensor_scalar_mul(
                        ctxn_bd[i * E:(i + 1) * E, s, i, :],
                        csum[i * E:(i + 1) * E, s, 0:E],
                        crec[i * E:(i + 1) * E, s:s + 1])

            # ------------- cross attention output + residual + LN2 -------
            cr_n = acts.tile([P, NT, DIM], F32, name="cr_n", tag="attn_n")
            for mt in range(NT):
                for hp in range(KF):
                    po = pp_s.tile([P, 2 * E], F32, name="co_ps", tag="small")
                    nc.tensor.matmul(
                        po[:], qscT[:, hp, mt, :],
                        ctxn_bd[:, hp, :, :].rearrange("p a e -> p (a e)"),
                        start=True, stop=True)
                    ev.add(cr_n[:, mt, hp * 2 * E:(hp + 1) * 2 * E], po[:],
                           ln1_n[:, mt, hp * 2 * E:(hp + 1) * 2 * E])

            ln2_n = acts.tile([P, NT, DIM], F32, name="ln2_n")
            _layernorm(nc, sbuf, eps_t, cr_n, g_bc["g2"], g_bc["b2"], ln2_n,
                       trivial)
            ln2T = acts.tile([P, KF, T], BF16, name="ln2T", tag="memT")
            for kf in range(KF):
                for mt in range(NT):
                    pt = pp_t.tile([P, P], F32, name="tpsum", tag="t128")
                    nc.tensor.transpose(pt[:], ln2_n[:, mt, kf * P:(kf + 1) * P],
                                        ident[:])
                    ev.copy(ln2T[:, kf, mt * P:(mt + 1) * P], pt[:])

            # ------------- FFN: stream W_ff1/W_ff2 in 1MB chunks ---------
            wff1_v = wff1_d.ap().rearrange("(k p) n -> p k n", p=P)
            wff2_v = wff2_d.ap().rearrange("(k p) n -> p k n", p=P)
            yps = [pp_b.tile([P, 512], F32, name=f"y_ps{mt}", tag="proj")
                   for mt in range(NT)]
            CH = 4  # ff blocks per streamed chunk
            for kc in range(NFF // CH):
                w1t = pw1.tile([P, KF, CH * P], BF16, name="w1s", tag="w1s")
                nc.sync.dma_start(
                    w1t[:], wff1_v[:, :, kc * CH * P:(kc + 1) * CH * P])
                w2t = pw2.tile([P, CH, 512], BF16, name="w2s", tag="w2s")
                nc.sync.dma_start(w2t[:], wff2_v[:, kc * CH:(kc + 1) * CH, :])
                for ki in range(CH):
                    kff = kc * CH + ki
                    h1ps = pp_t.tile([P, 512], F32, name="h1_ps", tag="t128")
                    for kf in range(KF):
                        nc.tensor.matmul(
                            h1ps[:], w1t[:, kf, ki * P:(ki + 1) * P],
                            ln2T[:, kf, :],
                            start=(kf == 0), stop=(kf == KF - 1))
                    h1t = ph1.tile([P, 512], BF16, name="h1s", tag="h1s")
                    if kff % 2 == 0:
                        nc.vector.tensor_scalar(h1t[:], h1ps[:],
                                                bff1T[:, kff:kff + 1], 0.0,
                                                ALU.add, ALU.max)
                    else:
                        nc.scalar.activation(h1t[:], h1ps[:], AF.Relu,
                                             bias=bff1T[:, kff:kff + 1])
                    for mt in range(NT):
                        nc.tensor.matmul(yps[mt][:],
                                         h1t[:, mt * P:(mt + 1) * P],
                                         w2t[:, ki, :],
                                         start=(kff == 0),
                                         stop=(kff == NFF - 1))

            y_n = acts.tile([P, NT, DIM], F32, name="y_n", tag="memn")
            for mt in range(NT):
                if trivial:
                    nc.vector.tensor_add(y_n[:, mt, :], yps[mt][:],
                                         ln2_n[:, mt, :])
                else:
                    t = sbuf.tile([P, DIM], F32, name="ffn_t")
                    nc.vector.tensor_add(t[:], yps[mt][:], ln2_n[:, mt, :])
                    nc.vector.tensor_add(y_n[:, mt, :], t[:], bff2_bc[:])

            out_n = acts.tile([P, NT, DIM], F32, name="out_n", tag="res1")
            _layernorm(nc, sbuf, eps_t, y_n, g_bc["g3"], g_bc["b3"], out_n,
                       trivial)
            nc.sync.dma_start(out_d.ap().rearrange("(m p) n -> p m n", p=P),
                              out_n[:])

    nc.compile()
    return nc


_CACHE = {}
LAST_EXEC_NS = []


def _module(trivial):
    key = ("fused", trivial)
    if key not in _CACHE:
        _CACHE[key] = build_fused(trivial)
    return _CACHE[key]


def _is_trivial(inp):
    z = lambda k: not np.any(inp[k])
    one = lambda k: bool(np.all(inp[k] == 1.0))
    return (z("b_qvk") and z("b_kv") and z("b_q") and z("b_ff2")
            and one("ln1_g") and z("ln1_b") and one("ln2_g") and z("ln2_b")
            and one("ln3_g") and z("ln3_b"))


def kernel(**inputs):
    inp = {k: np.ascontiguousarray(np.asarray(v)) for k, v in inputs.items()}
    trivial = _is_trivial(inp)
    m = _module(trivial)

    import ml_dtypes
    bf16 = ml_dtypes.bfloat16
    shared = {"W_qvk": inp["W_qvk"].astype(bf16),
              "W_kv": inp["W_kv"].astype(bf16),
              "W_q": inp["W_q"].astype(bf16),
              "W_ff1": inp["W_ff1"].astype(bf16),
              "W_ff2": inp["W_ff2"].astype(bf16),
              "b_ff1": inp["b_ff1"]}
    if not trivial:
        shared.update({k: inp[k] for k in (
            "b_qvk", "b_kv", "b_q", "b_ff2", "ln1_g", "ln1_b", "ln2_g",
            "ln2_b", "ln3_g", "ln3_b")})
    in_maps = []
    for c in range(N_CORES):
        b, q = c // 4, c % 4
        pmask = np.array([1.0 if j < q else 0.0 for j in range(4)],
                         np.float32)
        in_maps.append(dict(
            x=inp["x"][b, q * T:(q + 1) * T],
            mem=inp["memory"][b, q * T:(q + 1) * T],
            pmask=pmask,
            **shared,
        ))
    r = bass_utils.run_bass_kernel_spmd(m, in_maps, core_ids=list(range(N_CORES)))
    global LAST_EXEC_NS
    LAST_EXEC_NS = [r.exec_time_ns]

    out = np.zeros((B, N, DIM), np.float32)
    for c in range(N_CORES):
        b, q = c // 4, c % 4
        out[b, q * T:(q + 1) * T] = r.results[c]["out"]
    return out
